# revision 9
# baseline (speedup 1.0000x reference)
"""Trainium2 Bass kernel for nn_AGCRNCellWithMLP (AGCRN cell with per-node MLP weights).

Math (with nodes_ind == arange(N), which the harness guarantees):
    xh       = concat([x, h], -1)                      # [N, 129]
    combined = adj @ xh                                # [N, 129]
    r = sigmoid(mlp(combined, q, W_r, b_r))            # [N, 64]
    u = sigmoid(mlp(combined, q, W_u, b_u))
    h2 = r * h
    cand = tanh(mlp(concat([x, h2], -1), q, W_c, b_c))
    out = (1 - u) * h2 + u * cand
where mlp(v, q, W, b)[n, o] = sum_{d,i} q[n,d] v[n,i] W[d,i,o] + (q @ b)[n, o].

Sharding: data-parallel over nodes, 512 rows per core x 8 cores, no
collectives. All matmul operands fp16 (rel err ~3e-3 vs 2e-2 gate), PSUM and
final output fp32.

Measured PE facts this kernel is shaped around (N=moving cols, 2.4 GHz warm):
  - fp16 matmul with fresh 128-col weights: N/2.4 + ~210 ns (weight-swap stall)
  - fp16 matmul with <=64-col weights: N/2.4 (weight load fully hidden)
  - col-tiled matmuls on disjoint 32-col strips run CONCURRENTLY (~6ns apart)
So: every gate matmul uses 64-col weights; r|u pairs run concurrently on
strips {0,1}/{2,3}; the 129th-feature matmuls run as 4-way quads.

Per-core structure (full 512-node width, d-major gates):
  warmup  qbc_d[128,512] = q row d broadcast via K=1 matmul (ones x qrow),
          copied to fp16 SBUF on ScalarE; gate bias matmuls open gru/gc PSUM.
          Runs while adjT streams (only tiny DMAs needed).
  adj     combT = (adj @ xh[:,:128])^T: 32 matmuls (128-col xh weights pay the
          stall but hide under adj DMA); feature 128 via interleaved 1-col
          quad matmuls; quad partials reduced by the sel17b matmul.
  gates   per d: z_d = V (.) qbc_d on DVE (fp16), one concurrent r|u matmul
          pair (64-col weights, col strips) and for c one 64-col matmul.
          Tails: s[d,n]=q[d,n]*v128[n] resp. q[d,n]*x[n,64] + one K=16 matmul.
          Gate c feature order is [h2(64) | x(0:64)] so its tail x[:,64] is
          input-only and x DMAs straight into xh2T rows 64:128.
"""
import sys

sys.path.insert(0, "/opt/trn_rl_repo")

import numpy as np

N = 4096
IN = 64
QD = 16
CI = 2 * IN + 1          # 129
NCORES = 8
NS = N // NCORES         # 512 nodes per core
KT = N // 128            # 32 k-tiles for the adj matmul
CI2 = CI + 1             # xh slab width: 129 + pad col

_CACHE = {}


def build_nc():
    import concourse.bass as bass
    import concourse.bacc as bacc
    import concourse.tile as tile
    import concourse.mybir as mybir

    F32 = mybir.dt.float32
    F16 = mybir.dt.float16
    ACT = mybir.ActivationFunctionType

    nc = bacc.Bacc()
    dp = nc.declare_dram_parameter
    adjT_e = dp("adjT", [128, KT * NS], F16, isOutput=False)  # [p, (kt, n)]
    xh_e = dp("xh", [128, KT * CI2], F16, isOutput=False)     # [p, (kt, f)]
    qT_e = dp("qT", [QD, NS], F16, isOutput=False)
    qrows_e = dp("qrows", [1, QD * NS], F16, isOutput=False)
    ones1_e = dp("ones1", [1, 128], F16, isOutput=False)
    x64rep_e = dp("x64rep", [QD, NS], F16, isOutput=False)    # x[:,64] tiled 16
    xTc_e = dp("xTc", [IN, NS], F16, isOutput=False)          # x[:,0:64]^T
    hT_e = dp("hT", [IN, NS], F16, isOutput=False)
    wdru_e = dp("wdru", [128, QD * 2 * IN], F16, isOutput=False)  # [Wr_d|Wu_d]
    wdc_e = dp("wdc", [128, QD * IN], F16, isOutput=False)
    w128ru_e = dp("w128ru", [QD, 2 * IN], F16, isOutput=False)
    w128c_e = dp("w128c", [QD, IN], F16, isOutput=False)
    bru_e = dp("bru", [QD, 2 * IN], F16, isOutput=False)
    bc_e = dp("bc", [QD, IN], F16, isOutput=False)
    sel17b_e = dp("sel17b", [128, QD], F16, isOutput=False)
    out_e = dp("out", [IN, NS], F32, isOutput=True)

    with tile.TileContext(nc) as tc:
        with tc.tile_pool(name="const", bufs=1) as cpool, \
             tc.tile_pool(name="big", bufs=1) as bigpool, \
             tc.tile_pool(name="work", bufs=1) as wpool, \
             tc.tile_pool(name="zt", bufs=4) as ztpool, \
             tc.tile_pool(name="psM", bufs=1, space="PSUM") as psM, \
             tc.tile_pool(name="psQ", bufs=3, space="PSUM") as psQ:

            # ---- DMAs: tiny first (feed warmup), then xh+adjT stream -------
            qT = cpool.tile([QD, NS], F16, tag="qT")
            nc.sync.dma_start(qT[:], qT_e[:])
            qrows = cpool.tile([1, QD * NS], F16, tag="qrows")
            nc.sync.dma_start(qrows[:], qrows_e[:])
            ones1 = cpool.tile([1, 128], F16, tag="ones1")
            nc.sync.dma_start(ones1[:], ones1_e[:])
            bru = cpool.tile([QD, 2 * IN], F16, tag="bru")
            nc.sync.dma_start(bru[:], bru_e[:])
            bc = cpool.tile([QD, IN], F16, tag="bc")
            nc.sync.dma_start(bc[:], bc_e[:])
            sel17b = cpool.tile([128, QD], F16, tag="sel17b")
            nc.sync.dma_start(sel17b[:], sel17b_e[:])
            w128ru = cpool.tile([QD, 2 * IN], F16, tag="w128ru")
            nc.sync.dma_start(w128ru[:], w128ru_e[:])
            w128c = cpool.tile([QD, IN], F16, tag="w128c")
            nc.sync.dma_start(w128c[:], w128c_e[:])
            x64rep = cpool.tile([QD, NS], F16, tag="x64rep")
            nc.sync.dma_start(x64rep[:], x64rep_e[:])

            xh = bigpool.tile([128, KT * CI2], F16)
            adjT = bigpool.tile([128, KT * NS], F16)
            # xh2T rows 0:64 = h2 (written late), rows 64:128 = xT (DMA now)
            xh2T = wpool.tile([128, NS], F16, tag="xh2T")
            nc.sync.dma_start(xh2T[64:128, :], xTc_e[:])

            # interleave xh slabs with adjT slabs so matmul t never waits long
            for g in range(8):
                xlo, xhi = g * 4 * CI2, (g + 1) * 4 * CI2
                nc.sync.dma_start(xh[:, xlo:xhi], xh_e[:, xlo:xhi])
                alo, ahi = g * 4 * NS, (g + 1) * 4 * NS
                nc.sync.dma_start(adjT[:, alo:ahi], adjT_e[:, alo:ahi])
            # gate weights land during/after the adj phase (used later)
            wdru = cpool.tile([128, QD * 2 * IN], F16, tag="wdru")
            nc.sync.dma_start(wdru[:], wdru_e[:])
            hT = cpool.tile([IN, NS], F16, tag="hT")
            nc.sync.dma_start(hT[:], hT_e[:])
            wdc = cpool.tile([128, QD * IN], F16, tag="wdc")
            nc.sync.dma_start(wdc[:], wdc_e[:])

            # ---- warmup: qbc_d broadcasts + gate bias matmuls --------------
            gru = psM.tile([2 * IN, NS], F32, tag="gru")
            gc = psM.tile([IN, NS], F32, tag="gc")
            qbc = []
            for d in range(QD):
                ps = psQ.tile([128, NS], F32, tag="qb", name=f"qb{d}")
                nc.tensor.matmul(ps[:], ones1[:],
                                 qrows[0:1, d * NS:(d + 1) * NS],
                                 start=True, stop=True)
                qb = cpool.tile([128, NS], F16, tag=f"qbc{d}")
                nc.scalar.copy(qb[:], ps[:])
                qbc.append(qb)
            nc.tensor.matmul(gru[:], bru[:], qT[:],
                             start=True, stop=False, skip_group_check=True)
            nc.tensor.matmul(gc[:], bc[:], qT[:],
                             start=True, stop=False, skip_group_check=True)
            # s_c = q (.) x64 needs only inputs: do it early on DVE
            s_c = wpool.tile([QD, NS], F16, tag="s_c")
            nc.vector.tensor_mul(s_c[:], qT[:], x64rep[:])

            # ---- adj matmul: pc + interleaved pl quads ---------------------
            pc = psM.tile([128, NS], F32, tag="pc")
            pl = psM.tile([128, NS], F32, tag="pl")
            for g in range(8):
                for t in range(4 * g, 4 * g + 4):
                    nc.tensor.matmul(pc[:], xh[:, t * CI2: t * CI2 + 128],
                                     adjT[:, t * NS:(t + 1) * NS],
                                     start=(t == 0), stop=(t == KT - 1),
                                     skip_group_check=True)
                for t in range(4 * g, 4 * g + 4):
                    j = t % 4
                    nc.tensor.matmul(pl[32 * j:32 * j + 1, :],
                                     xh[:, t * CI2 + 128: t * CI2 + 129],
                                     adjT[:, t * NS:(t + 1) * NS],
                                     start=(g == 0), stop=(g == 7),
                                     tile_position=(0, 32 * j),
                                     skip_group_check=True)
            combT = wpool.tile([128, NS], F16, tag="combT")
            nc.scalar.copy(combT[:], pc[:])
            pl_sb = wpool.tile([128, NS], F16, tag="pl_sb")
            nc.scalar.copy(pl_sb[:], pl[:])
            v128 = psQ.tile([QD, NS], F32, tag="qb", name="v128")
            nc.tensor.matmul(v128[:], sel17b[:], pl_sb[:], start=True, stop=True)
            s_ru = wpool.tile([QD, NS], F16, tag="s_ru")
            nc.vector.tensor_mul(s_ru[:], qT[:], v128[:])

            # ---- gates r, u: concurrent 64-col pairs on strips {0,1}/{2,3} -
            for d in range(QD):
                z = ztpool.tile([128, NS], F16, tag="z", name=f"zru{d}")
                nc.vector.tensor_mul(z[:], combT[:], qbc[d][:])
                c0 = d * 2 * IN
                nc.tensor.matmul(gru[0:IN, :], wdru[:, c0:c0 + IN], z[:],
                                 start=False, stop=False,
                                 tile_position=(0, 0), skip_group_check=True)
                nc.tensor.matmul(gru[IN:2 * IN, :], wdru[:, c0 + IN:c0 + 2 * IN],
                                 z[:], start=False, stop=False,
                                 tile_position=(0, 64), skip_group_check=True)
            nc.tensor.matmul(gru[0:IN, :], w128ru[:, 0:IN], s_ru[:],
                             start=False, stop=True,
                             tile_position=(0, 0), skip_group_check=True)
            nc.tensor.matmul(gru[IN:2 * IN, :], w128ru[:, IN:2 * IN], s_ru[:],
                             start=False, stop=True,
                             tile_position=(0, 64), skip_group_check=True)
            r_sb = wpool.tile([IN, NS], F16, tag="r_sb")
            nc.scalar.activation(r_sb[:], gru[0:IN, :], ACT.Sigmoid)
            u_sb = wpool.tile([IN, NS], F16, tag="u_sb")
            nc.scalar.activation(u_sb[:], gru[IN:2 * IN, :], ACT.Sigmoid)

            # h2 = r*h -> xh2T rows 0:64
            nc.vector.tensor_mul(xh2T[0:IN, :], r_sb[:], hT[:])

            # ---- gate c (d-major over [h2 | x]) ----------------------------
            for d in range(QD):
                z = ztpool.tile([128, NS], F16, tag="z", name=f"zc{d}")
                nc.vector.tensor_mul(z[:], xh2T[:], qbc[d][:])
                nc.tensor.matmul(gc[:], wdc[:, d * IN:(d + 1) * IN],
                                 z[:], start=False, stop=False,
                                 skip_group_check=True)
            nc.tensor.matmul(gc[:], w128c[:], s_c[:],
                             start=False, stop=True, skip_group_check=True)
            cand = wpool.tile([IN, NS], F16, tag="cand")
            nc.scalar.activation(cand[:], gc[:], ACT.Tanh)

            # ---- out = h2 + u*(cand - h2) ----------------------------------
            dt_ = wpool.tile([IN, NS], F16, tag="dt")
            nc.vector.tensor_sub(dt_[:], cand[:], xh2T[0:IN, :])
            et = wpool.tile([IN, NS], F16, tag="et")
            nc.vector.tensor_mul(et[:], u_sb[:], dt_[:])
            outT = wpool.tile([IN, NS], F32, tag="outT")
            nc.vector.tensor_add(outT[:], xh2T[0:IN, :], et[:])
            nc.sync.dma_start(out_e[:], outT[:])
    nc.compile()
    return nc


def _f16(a):
    return np.ascontiguousarray(np.asarray(a, np.float16))


def prep_in_maps(x, h, query_vectors, adj, nodes_ind, W_r, b_r, W_u, b_u, W_c, b_c):
    x = np.asarray(x, np.float32)
    h = np.asarray(h, np.float32)
    q = np.asarray(query_vectors, np.float32)
    adj = np.asarray(adj, np.float32)
    ni = np.asarray(nodes_ind)
    assert np.array_equal(ni, np.arange(N)), "kernel assumes nodes_ind == arange(N)"

    xh = np.concatenate([x, h, np.zeros((N, 1), np.float32)], axis=-1)  # [N,130]
    xh_sb = _f16(xh.reshape(KT, 128, CI2).transpose(1, 0, 2).reshape(128, KT * CI2))

    # d-major weight blocks. ru: block d = [W_r[d, i<128] | W_u[d, i<128]],
    # tail (i=128) separate. c: feature order [h2 (orig i 65..128) | x (0..63)],
    # tail = orig i 64 (x[:,64]).
    Wr = np.asarray(W_r, np.float32)
    Wu = np.asarray(W_u, np.float32)
    Wc = np.asarray(W_c, np.float32)
    wdru = np.concatenate([Wr[:, :128, :], Wu[:, :128, :]], axis=2)  # [16,128,128]
    wdru = _f16(wdru.transpose(1, 0, 2).reshape(128, QD * 2 * IN))
    perm_c = list(range(65, CI)) + list(range(0, 64))                # [h2|x]
    wdc = Wc[:, perm_c, :]                                           # [16,128,64]
    wdc = _f16(wdc.transpose(1, 0, 2).reshape(128, QD * IN))
    w128ru = _f16(np.concatenate([Wr[:, 128, :], Wu[:, 128, :]], axis=1))
    w128c = _f16(Wc[:, 64, :])
    bru = _f16(np.concatenate([np.asarray(b_r, np.float32),
                               np.asarray(b_u, np.float32)], axis=1))
    bc = _f16(np.asarray(b_c, np.float32))

    sel17b = np.zeros((128, QD), np.float32)
    for jj in range(4):
        sel17b[32 * jj, :] = 1.0
    ones1 = np.ones((1, 128), np.float32)

    in_maps = []
    for c in range(NCORES):
        s = slice(c * NS, (c + 1) * NS)
        adjT_sb = _f16(adj[s].T.reshape(KT, 128, NS).transpose(1, 0, 2)
                       .reshape(128, KT * NS))
        in_maps.append({
            "adjT": adjT_sb,
            "xh": xh_sb,
            "qT": _f16(q[s].T),
            "qrows": _f16(q[s].T.reshape(1, QD * NS)),
            "ones1": _f16(ones1),
            "x64rep": _f16(np.tile(x[s, 64], (QD, 1))),
            "xTc": _f16(x[s, 0:64].T),
            "hT": _f16(h[s].T),
            "wdru": wdru, "wdc": wdc,
            "w128ru": w128ru, "w128c": w128c,
            "bru": bru, "bc": bc,
            "sel17b": _f16(sel17b),
        })
    return in_maps


def kernel(**inputs):
    from concourse.bass_utils import run_bass_kernel_spmd

    if "nc" not in _CACHE:
        _CACHE["nc"] = build_nc()
    nc = _CACHE["nc"]
    in_maps = prep_in_maps(**inputs)
    res = run_bass_kernel_spmd(nc, in_maps, core_ids=list(range(NCORES)))
    out = np.empty((N, IN), np.float32)
    for c in range(NCORES):
        out[c * NS:(c + 1) * NS, :] = res.results[c]["out"].T
    return out


# revision 10
# speedup vs baseline: 1.0946x; 1.0946x over previous
"""Trainium2 Bass kernel for nn_AGCRNCellWithMLP (AGCRN cell with per-node MLP weights).

Math (with nodes_ind == arange(N), which the harness guarantees):
    xh       = concat([x, h], -1)                      # [N, 129]
    combined = adj @ xh                                # [N, 129]
    r = sigmoid(mlp(combined, q, W_r, b_r))            # [N, 64]
    u = sigmoid(mlp(combined, q, W_u, b_u))
    h2 = r * h
    cand = tanh(mlp(concat([x, h2], -1), q, W_c, b_c))
    out = (1 - u) * h2 + u * cand
where mlp(v, q, W, b)[n, o] = sum_{d,i} q[n,d] v[n,i] W[d,i,o] + (q @ b)[n, o].

Sharding: data-parallel over nodes, 512 rows per core x 8 cores, no
collectives. All matmul operands fp16 (rel err ~3e-3 vs 2e-2 gate), PSUM and
final output fp32.

Measured HW facts this kernel is shaped around:
  - fp16 matmul, fresh 128-col weights: N/2.4 + ~210 ns (weight-swap stall);
    <=64-col weights or 1-col: N/2.4 (hidden). Col-tiled quads on disjoint
    strips run concurrently.
  - DVE fp16 tensor_tensor: full-tile [128,1024] = 677 ns; SLICED inputs are
    3x slower, so all elementwise z work uses full-width paired tiles.
  - ~8 us fixed engine-preamble before the first matmul; DMA streams ~358 GB/s.

Per-core structure (full 512-node width, d-major gates):
  adj     combT = (adj @ xh[:,:128])^T: 32 matmuls (xh k-tile weights),
          feature 128 via interleaved 1-col 4-way quad matmuls, quad partials
          reduced by the sel17b matmul into v128rep.
  gates   qbc pairs [q_d | q_d+1] broadcast to 128 rows come FROM THE HOST
          (DMA'd during the adj phase). Per pair: one full-width DVE mul
          z2 = V2 (.) qbcpair (V2 = [combT|combT] or [xh2|xh2]), then two
          accumulating matmuls G += Wd^T @ z2-half. Tail feature (i=128):
          s[d,n] = q[d,n]*v128[n] resp. q[d,n]*x[n,64], one K=16 matmul each.
          Gate c feature order is [h2(64) | x(0:64)] so its tail x[:,64] is
          input-only; x DMAs straight into xh2T2 rows 64:128.
"""
import sys

sys.path.insert(0, "/opt/trn_rl_repo")

import numpy as np

N = 4096
IN = 64
QD = 16
CI = 2 * IN + 1          # 129
NCORES = 8
NS = N // NCORES         # 512 nodes per core
NS2 = 2 * NS             # paired width
KT = N // 128            # 32 k-tiles for the adj matmul
CI2 = CI + 1             # xh slab width: 129 + pad col

_CACHE = {}


def build_nc():
    import concourse.bass as bass
    import concourse.bacc as bacc
    import concourse.tile as tile
    import concourse.mybir as mybir

    F32 = mybir.dt.float32
    F16 = mybir.dt.float16
    ACT = mybir.ActivationFunctionType

    nc = bacc.Bacc()
    dp = nc.declare_dram_parameter
    adjT_e = dp("adjT", [128, KT * NS], F16, isOutput=False)  # [p, (kt, n)]
    xh_e = dp("xh", [128, KT * CI2], F16, isOutput=False)     # [p, (kt, f)]
    qT_e = dp("qT", [QD, NS], F16, isOutput=False)
    qbp_e = dp("qbp", [128, 8 * NS2], F16, isOutput=False)    # [qbc_d|qbc_d+1] x8
    x64rep_e = dp("x64rep", [QD, NS], F16, isOutput=False)    # x[:,64] tiled 16
    xTc_e = dp("xTc", [IN, NS], F16, isOutput=False)          # x[:,0:64]^T
    hT_e = dp("hT", [IN, NS], F16, isOutput=False)
    wdru_e = dp("wdru", [128, QD * 2 * IN], F16, isOutput=False)  # [Wr_d|Wu_d]
    wdc_e = dp("wdc", [128, QD * IN], F16, isOutput=False)
    w128ru_e = dp("w128ru", [QD, 2 * IN], F16, isOutput=False)
    w128c_e = dp("w128c", [QD, IN], F16, isOutput=False)
    bru_e = dp("bru", [QD, 2 * IN], F16, isOutput=False)
    bc_e = dp("bc", [QD, IN], F16, isOutput=False)
    sel17b_e = dp("sel17b", [128, QD], F16, isOutput=False)
    out_e = dp("out", [IN, NS], F32, isOutput=True)

    with tile.TileContext(nc) as tc:
        with tc.tile_pool(name="const", bufs=1) as cpool, \
             tc.tile_pool(name="big", bufs=1) as bigpool, \
             tc.tile_pool(name="work", bufs=1) as wpool, \
             tc.tile_pool(name="zt", bufs=3) as ztpool, \
             tc.tile_pool(name="psM", bufs=1, space="PSUM") as psM, \
             tc.tile_pool(name="psQ", bufs=1, space="PSUM") as psQ:

            # ---- DMAs ------------------------------------------------------
            qT = cpool.tile([QD, NS], F16, tag="qT")
            nc.sync.dma_start(qT[:], qT_e[:])
            bru = cpool.tile([QD, 2 * IN], F16, tag="bru")
            nc.sync.dma_start(bru[:], bru_e[:])
            bc = cpool.tile([QD, IN], F16, tag="bc")
            nc.sync.dma_start(bc[:], bc_e[:])
            sel17b = cpool.tile([128, QD], F16, tag="sel17b")
            nc.sync.dma_start(sel17b[:], sel17b_e[:])
            w128ru = cpool.tile([QD, 2 * IN], F16, tag="w128ru")
            nc.sync.dma_start(w128ru[:], w128ru_e[:])
            w128c = cpool.tile([QD, IN], F16, tag="w128c")
            nc.sync.dma_start(w128c[:], w128c_e[:])
            x64rep = cpool.tile([QD, NS], F16, tag="x64rep")
            nc.sync.dma_start(x64rep[:], x64rep_e[:])

            xh = bigpool.tile([128, KT * CI2], F16)
            adjT = bigpool.tile([128, KT * NS], F16)
            # xh2T2 = [[h2|x], [h2|x]] paired: x -> rows 64:128 of both halves
            xh2T2 = wpool.tile([128, NS2], F16, tag="xh2T2")
            nc.sync.dma_start(xh2T2[64:128, 0:NS], xTc_e[:])
            nc.sync.dma_start(xh2T2[64:128, NS:NS2], xTc_e[:])

            # interleave xh slabs with adjT slabs so matmul t never waits long
            for g in range(8):
                xlo, xhi = g * 4 * CI2, (g + 1) * 4 * CI2
                nc.sync.dma_start(xh[:, xlo:xhi], xh_e[:, xlo:xhi])
                alo, ahi = g * 4 * NS, (g + 1) * 4 * NS
                nc.sync.dma_start(adjT[:, alo:ahi], adjT_e[:, alo:ahi])
            # gate-phase tensors land during/after the adj phase, in use order
            wdru = cpool.tile([128, QD * 2 * IN], F16, tag="wdru")
            nc.sync.dma_start(wdru[:], wdru_e[:])
            qbp = cpool.tile([128, 8 * NS2], F16, tag="qbp")
            for j in range(8):
                nc.sync.dma_start(qbp[:, j * NS2:(j + 1) * NS2],
                                  qbp_e[:, j * NS2:(j + 1) * NS2])
            hT = cpool.tile([IN, NS], F16, tag="hT")
            nc.sync.dma_start(hT[:], hT_e[:])
            wdc = cpool.tile([128, QD * IN], F16, tag="wdc")
            nc.sync.dma_start(wdc[:], wdc_e[:])

            # ---- gate bias matmuls open the PSUM accumulations -------------
            gru = psM.tile([2 * IN, NS], F32, tag="gru")
            gc = psM.tile([IN, NS], F32, tag="gc")
            nc.tensor.matmul(gru[:], bru[:], qT[:],
                             start=True, stop=False, skip_group_check=True)
            nc.tensor.matmul(gc[:], bc[:], qT[:],
                             start=True, stop=False, skip_group_check=True)
            # s_c = q (.) x64 needs only inputs: do it early on DVE
            s_c = wpool.tile([QD, NS], F16, tag="s_c")
            nc.vector.tensor_mul(s_c[:], qT[:], x64rep[:])

            # ---- adj matmul: pc + interleaved pl quads ---------------------
            pc = psM.tile([128, NS], F32, tag="pc")
            pl = psM.tile([128, NS], F32, tag="pl")
            for g in range(8):
                for t in range(4 * g, 4 * g + 4):
                    nc.tensor.matmul(pc[:], xh[:, t * CI2: t * CI2 + 128],
                                     adjT[:, t * NS:(t + 1) * NS],
                                     start=(t == 0), stop=(t == KT - 1),
                                     skip_group_check=True)
                for t in range(4 * g, 4 * g + 4):
                    j = t % 4
                    nc.tensor.matmul(pl[32 * j:32 * j + 1, :],
                                     xh[:, t * CI2 + 128: t * CI2 + 129],
                                     adjT[:, t * NS:(t + 1) * NS],
                                     start=(g == 0), stop=(g == 7),
                                     tile_position=(0, 32 * j),
                                     skip_group_check=True)
            combT2 = wpool.tile([128, NS2], F16, tag="combT2")
            nc.scalar.copy(combT2[:, 0:NS], pc[:])
            nc.scalar.copy(combT2[:, NS:NS2], pc[:])
            pl_sb = wpool.tile([128, NS], F16, tag="pl_sb")
            nc.scalar.copy(pl_sb[:], pl[:])
            v128 = psQ.tile([QD, NS], F32, tag="v128")
            nc.tensor.matmul(v128[:], sel17b[:], pl_sb[:], start=True, stop=True)
            s_ru = wpool.tile([QD, NS], F16, tag="s_ru")
            nc.vector.tensor_mul(s_ru[:], qT[:], v128[:])

            # ---- gates r, u (d-major, paired z) ----------------------------
            for j in range(8):
                z2 = ztpool.tile([128, NS2], F16, tag="z", name=f"zru{j}")
                nc.vector.tensor_mul(z2[:], combT2[:], qbp[:, j * NS2:(j + 1) * NS2])
                for k in range(2):
                    d = 2 * j + k
                    nc.tensor.matmul(gru[:], wdru[:, d * 2 * IN:(d + 1) * 2 * IN],
                                     z2[:, k * NS:(k + 1) * NS],
                                     start=False, stop=False,
                                     skip_group_check=True)
            nc.tensor.matmul(gru[:], w128ru[:], s_ru[:],
                             start=False, stop=True, skip_group_check=True)
            r_sb = wpool.tile([IN, NS], F16, tag="r_sb")
            nc.scalar.activation(r_sb[:], gru[0:IN, :], ACT.Sigmoid)
            u_sb = wpool.tile([IN, NS], F16, tag="u_sb")
            nc.scalar.activation(u_sb[:], gru[IN:2 * IN, :], ACT.Sigmoid)

            # h2 = r*h -> xh2T2 rows 0:64 (both halves)
            nc.vector.tensor_mul(xh2T2[0:IN, 0:NS], r_sb[:], hT[:])
            nc.vector.tensor_mul(xh2T2[0:IN, NS:NS2], r_sb[:], hT[:])

            # ---- gate c (d-major over [h2 | x], paired z) ------------------
            for j in range(8):
                z2 = ztpool.tile([128, NS2], F16, tag="z", name=f"zc{j}")
                nc.vector.tensor_mul(z2[:], xh2T2[:], qbp[:, j * NS2:(j + 1) * NS2])
                for k in range(2):
                    d = 2 * j + k
                    nc.tensor.matmul(gc[:], wdc[:, d * IN:(d + 1) * IN],
                                     z2[:, k * NS:(k + 1) * NS],
                                     start=False, stop=False,
                                     skip_group_check=True)
            nc.tensor.matmul(gc[:], w128c[:], s_c[:],
                             start=False, stop=True, skip_group_check=True)
            cand = wpool.tile([IN, NS], F16, tag="cand")
            nc.scalar.activation(cand[:], gc[:], ACT.Tanh)

            # ---- out = h2 + u*(cand - h2) ----------------------------------
            dt_ = wpool.tile([IN, NS], F16, tag="dt")
            nc.vector.tensor_sub(dt_[:], cand[:], xh2T2[0:IN, 0:NS])
            et = wpool.tile([IN, NS], F16, tag="et")
            nc.vector.tensor_mul(et[:], u_sb[:], dt_[:])
            outT = wpool.tile([IN, NS], F32, tag="outT")
            nc.vector.tensor_add(outT[:], xh2T2[0:IN, 0:NS], et[:])
            nc.sync.dma_start(out_e[:], outT[:])
    nc.compile()
    return nc


def _f16(a):
    return np.ascontiguousarray(np.asarray(a, np.float16))


def prep_in_maps(x, h, query_vectors, adj, nodes_ind, W_r, b_r, W_u, b_u, W_c, b_c):
    x = np.asarray(x, np.float32)
    h = np.asarray(h, np.float32)
    q = np.asarray(query_vectors, np.float32)
    adj = np.asarray(adj, np.float32)
    ni = np.asarray(nodes_ind)
    assert np.array_equal(ni, np.arange(N)), "kernel assumes nodes_ind == arange(N)"

    xh = np.concatenate([x, h, np.zeros((N, 1), np.float32)], axis=-1)  # [N,130]
    xh_sb = _f16(xh.reshape(KT, 128, CI2).transpose(1, 0, 2).reshape(128, KT * CI2))

    Wr = np.asarray(W_r, np.float32)
    Wu = np.asarray(W_u, np.float32)
    Wc = np.asarray(W_c, np.float32)
    wdru = np.concatenate([Wr[:, :128, :], Wu[:, :128, :]], axis=2)  # [16,128,128]
    wdru = _f16(wdru.transpose(1, 0, 2).reshape(128, QD * 2 * IN))
    perm_c = list(range(65, CI)) + list(range(0, 64))                # [h2|x]
    wdc = Wc[:, perm_c, :]                                           # [16,128,64]
    wdc = _f16(wdc.transpose(1, 0, 2).reshape(128, QD * IN))
    w128ru = _f16(np.concatenate([Wr[:, 128, :], Wu[:, 128, :]], axis=1))
    w128c = _f16(Wc[:, 64, :])
    bru = _f16(np.concatenate([np.asarray(b_r, np.float32),
                               np.asarray(b_u, np.float32)], axis=1))
    bc = _f16(np.asarray(b_c, np.float32))

    sel17b = np.zeros((128, QD), np.float32)
    for jj in range(4):
        sel17b[32 * jj, :] = 1.0

    in_maps = []
    for c in range(NCORES):
        s = slice(c * NS, (c + 1) * NS)
        adjT_sb = _f16(adj[s].T.reshape(KT, 128, NS).transpose(1, 0, 2)
                       .reshape(128, KT * NS))
        # qbc pairs: [128, 8*1024]; pair j = [q_2j bcast | q_2j+1 bcast]
        qs = q[s].T                                    # [16, 512]
        qbp = np.broadcast_to(qs[:, None, :], (QD, 128, NS)).reshape(QD, 128, NS)
        qbp = qbp.transpose(1, 0, 2).reshape(128, QD * NS)
        in_maps.append({
            "adjT": adjT_sb,
            "xh": xh_sb,
            "qT": _f16(qs),
            "qbp": _f16(qbp),
            "x64rep": _f16(np.tile(x[s, 64], (QD, 1))),
            "xTc": _f16(x[s, 0:64].T),
            "hT": _f16(h[s].T),
            "wdru": wdru, "wdc": wdc,
            "w128ru": w128ru, "w128c": w128c,
            "bru": bru, "bc": bc,
            "sel17b": _f16(sel17b),
        })
    return in_maps


def kernel(**inputs):
    from concourse.bass_utils import run_bass_kernel_spmd

    if "nc" not in _CACHE:
        _CACHE["nc"] = build_nc()
    nc = _CACHE["nc"]
    in_maps = prep_in_maps(**inputs)
    res = run_bass_kernel_spmd(nc, in_maps, core_ids=list(range(NCORES)))
    out = np.empty((N, IN), np.float32)
    for c in range(NCORES):
        out[c * NS:(c + 1) * NS, :] = res.results[c]["out"].T
    return out


# revision 11
# speedup vs baseline: 1.2788x; 1.1683x over previous
"""Trainium2 Bass kernel for nn_AGCRNCellWithMLP (AGCRN cell with per-node MLP weights).

Math (with nodes_ind == arange(N), which the harness guarantees):
    xh       = concat([x, h], -1)                      # [N, 129]
    combined = adj @ xh                                # [N, 129]
    r = sigmoid(mlp(combined, q, W_r, b_r))            # [N, 64]
    u = sigmoid(mlp(combined, q, W_u, b_u))
    h2 = r * h
    cand = tanh(mlp(concat([x, h2], -1), q, W_c, b_c))
    out = (1 - u) * h2 + u * cand
where mlp(v, q, W, b)[n, o] = sum_{d,i} q[n,d] v[n,i] W[d,i,o] + (q @ b)[n, o].

Sharding: data-parallel over nodes, 512 rows per core x 8 cores, no
collectives. All matmul operands fp16 (rel err ~3e-3 vs 2e-2 gate), PSUM and
final output fp32.

Measured HW facts this kernel is shaped around:
  - fp16 matmul, fresh 128-col weights: +~210ns weight-swap stall; <=64-col
    hidden. Col-tiled quads on disjoint strips run concurrently.
  - DVE fp16 TT: full-tile [128,1024] = 677ns; sliced inputs 3x slower.
  - Each dma_start costs ~650ns on the Sync queue: batch DMAs aggressively.
  - ~8us fixed engine preamble; (1-u) comes free as sigmoid(-gru_u).

Per-core structure (full 512-node width, d-major gates):
  blobA   one DMA with every small constant packed [128, *] column-wise.
  stream  one interleaved blob [xh-slabs | adjT-slabs] x 8 chunk-DMAs;
          combT = (adj @ xh)^T via 32 matmuls + 1-col quad matmuls for
          feature 128 (quad partials reduced by the sel17b matmul).
  gates   qbc pairs [q_2j|q_2j+1] broadcast to 128 rows DMA'd from host (2
          triggers). Per pair: one full-width DVE mul z2 = V2 (.) qbcpair,
          two accumulating matmuls. Tail feature (i=128) via s-vectors and
          one K=16 matmul per gate group. Gate c feature order [h2|x(0:64)]
          puts its tail on the input-only x[:,64].
  out     u' = sigmoid(-gru_u) early; out = u'*h2 + u*cand.
"""
import sys

sys.path.insert(0, "/opt/trn_rl_repo")

import numpy as np

N = 4096
IN = 64
QD = 16
CI = 2 * IN + 1          # 129
NCORES = 8
NS = N // NCORES         # 512 nodes per core
NS2 = 2 * NS             # paired width
KT = N // 128            # 32 k-tiles for the adj matmul
CI2 = CI + 1             # xh slab width: 129 + pad col
CW = CI2 + NS            # stream chunk width per k-tile (xh slab + adjT slab)

# blobA column offsets (all fp16, packed [128, BW])
_OFF = {}
_cols = 0
for _name, _w in [("qT", NS), ("bru", 2 * IN), ("bc", IN), ("sel17b", QD),
                  ("w128ru", 2 * IN), ("w128c", IN), ("x64rep", NS),
                  ("xTc", NS), ("hT", NS)]:
    _OFF[_name] = _cols
    _cols += _w
BW = _cols

_CACHE = {}


def build_nc():
    import concourse.bass as bass
    import concourse.bacc as bacc
    import concourse.tile as tile
    import concourse.mybir as mybir

    F32 = mybir.dt.float32
    F16 = mybir.dt.float16
    ACT = mybir.ActivationFunctionType

    nc = bacc.Bacc()
    dp = nc.declare_dram_parameter
    blobA_e = dp("blobA", [128, BW], F16, isOutput=False)
    strm_e = dp("strm", [128, KT * CW], F16, isOutput=False)  # [xh_t | adjT_t] x32
    qbp_e = dp("qbp", [128, 8 * NS2], F16, isOutput=False)    # [qbc_2j|qbc_2j+1] x8
    wdru_e = dp("wdru", [128, QD * 2 * IN], F16, isOutput=False)
    wdc_e = dp("wdc", [128, QD * IN], F16, isOutput=False)
    out_e = dp("out", [IN, NS], F32, isOutput=True)

    with tile.TileContext(nc) as tc:
        with tc.tile_pool(name="const", bufs=1) as cpool, \
             tc.tile_pool(name="big", bufs=1) as bigpool, \
             tc.tile_pool(name="work", bufs=1) as wpool, \
             tc.tile_pool(name="zt", bufs=3) as ztpool, \
             tc.tile_pool(name="psM", bufs=1, space="PSUM") as psM, \
             tc.tile_pool(name="psQ", bufs=1, space="PSUM") as psQ:

            # ---- DMAs ------------------------------------------------------
            blobA = cpool.tile([128, BW], F16, tag="blobA")
            nc.sync.dma_start(blobA[:], blobA_e[:])

            def cslice(name, w, p=128):
                o = _OFF[name]
                return blobA[0:p, o:o + w]

            qT = cslice("qT", NS, QD)
            bru = cslice("bru", 2 * IN, QD)
            bc = cslice("bc", IN, QD)
            sel17b = cslice("sel17b", QD, 128)
            w128ru = cslice("w128ru", 2 * IN, QD)
            w128c = cslice("w128c", IN, QD)
            x64rep = cslice("x64rep", NS, QD)
            xTc = cslice("xTc", NS, IN)
            hT = cslice("hT", NS, IN)

            strm = bigpool.tile([128, KT * CW], F16)
            for g in range(8):
                lo, hi = g * 4 * CW, (g + 1) * 4 * CW
                nc.sync.dma_start(strm[:, lo:hi], strm_e[:, lo:hi])
            wdru = cpool.tile([128, QD * 2 * IN], F16, tag="wdru")
            nc.sync.dma_start(wdru[:], wdru_e[:])
            qbp = cpool.tile([128, 8 * NS2], F16, tag="qbp")
            nc.sync.dma_start(qbp[:, 0:4 * NS2], qbp_e[:, 0:4 * NS2])
            nc.sync.dma_start(qbp[:, 4 * NS2:8 * NS2], qbp_e[:, 4 * NS2:8 * NS2])
            wdc = cpool.tile([128, QD * IN], F16, tag="wdc")
            nc.sync.dma_start(wdc[:], wdc_e[:])

            def xh_t(t, a, b):
                return strm[:, t * CW + a: t * CW + b]

            def adj_t(t):
                return strm[:, t * CW + CI2:(t + 1) * CW]

            # xh2T2 = [[h2|x],[h2|x]]: x copied into rows 64:128 on ScalarE
            xh2T2 = wpool.tile([128, NS2], F16, tag="xh2T2")
            nc.scalar.copy(xh2T2[64:128, 0:NS], xTc)
            nc.scalar.copy(xh2T2[64:128, NS:NS2], xTc)
            # s_c = q (.) x64, needs only inputs
            s_c = wpool.tile([QD, NS], F16, tag="s_c")
            nc.vector.tensor_mul(s_c[:], qT, x64rep)

            # ---- gate bias matmuls open the PSUM accumulations -------------
            gru = psM.tile([2 * IN, NS], F32, tag="gru")
            gc = psM.tile([IN, NS], F32, tag="gc")
            nc.tensor.matmul(gru[:], bru, qT,
                             start=True, stop=False, skip_group_check=True)
            nc.tensor.matmul(gc[:], bc, qT,
                             start=True, stop=False, skip_group_check=True)

            # ---- adj matmul: pc + interleaved pl quads ---------------------
            pc = psM.tile([128, NS], F32, tag="pc")
            pl = psM.tile([128, NS], F32, tag="pl")
            for g in range(8):
                for t in range(4 * g, 4 * g + 4):
                    nc.tensor.matmul(pc[:], xh_t(t, 0, 128), adj_t(t),
                                     start=(t == 0), stop=(t == KT - 1),
                                     skip_group_check=True)
                for t in range(4 * g, 4 * g + 4):
                    j = t % 4
                    nc.tensor.matmul(pl[32 * j:32 * j + 1, :],
                                     xh_t(t, 128, 129), adj_t(t),
                                     start=(g == 0), stop=(g == 7),
                                     tile_position=(0, 32 * j),
                                     skip_group_check=True)
            combT2 = wpool.tile([128, NS2], F16, tag="combT2")
            nc.scalar.copy(combT2[:, 0:NS], pc[:])
            nc.vector.tensor_copy(combT2[:, NS:NS2], pc[:])

            # ---- gates r, u (d-major, paired z) ----------------------------
            def z2_mul(j, V2, name):
                z2 = ztpool.tile([128, NS2], F16, tag="z", name=name)
                nc.vector.tensor_mul(z2[:], V2[:], qbp[:, j * NS2:(j + 1) * NS2])
                return z2

            def wf_pair(j, z2, ps, w, m):
                for k in range(2):
                    d = 2 * j + k
                    nc.tensor.matmul(ps, w[:, d * m:(d + 1) * m],
                                     z2[:, k * NS:(k + 1) * NS],
                                     start=False, stop=False,
                                     skip_group_check=True)

            for j in range(8):
                z2 = z2_mul(j, combT2, f"zru{j}")
                wf_pair(j, z2, gru[:], wdru, 2 * IN)
                if j == 0:
                    # tail inputs, off the critical path of the first pairs
                    pl_sb = wpool.tile([128, NS], F16, tag="pl_sb")
                    nc.scalar.copy(pl_sb[:], pl[:])
            v128 = psQ.tile([QD, NS], F32, tag="v128")
            nc.tensor.matmul(v128[:], sel17b, pl_sb[:], start=True, stop=True)
            s_ru = wpool.tile([QD, NS], F16, tag="s_ru")
            nc.vector.tensor_mul(s_ru[:], qT, v128[:])
            nc.tensor.matmul(gru[:], w128ru, s_ru[:],
                             start=False, stop=True, skip_group_check=True)
            r_sb = wpool.tile([IN, NS], F16, tag="r_sb")
            nc.scalar.activation(r_sb[:], gru[0:IN, :], ACT.Sigmoid)

            # h2 = r*h -> xh2T2 rows 0:64 (both halves)
            nc.vector.tensor_mul(xh2T2[0:IN, 0:NS], r_sb[:], hT)
            nc.vector.tensor_mul(xh2T2[0:IN, NS:NS2], r_sb[:], hT)

            # ---- gate c (d-major over [h2 | x], paired z) ------------------
            for j in range(8):
                z2 = z2_mul(j, xh2T2, f"zc{j}")
                wf_pair(j, z2, gc[:], wdc, IN)
                if j == 0:
                    # u and u' = 1-u on ScalarE while DVE/PE run the c loop
                    u_sb = wpool.tile([IN, NS], F16, tag="u_sb")
                    nc.scalar.activation(u_sb[:], gru[IN:2 * IN, :], ACT.Sigmoid)
                    up_sb = wpool.tile([IN, NS], F16, tag="up_sb")
                    nc.scalar.activation(up_sb[:], gru[IN:2 * IN, :],
                                         ACT.Sigmoid, scale=-1.0)
                if j == 1:
                    # e2 = (1-u)*h2, also early
                    e2 = wpool.tile([IN, NS], F16, tag="e2")
                    nc.vector.tensor_mul(e2[:], up_sb[:], xh2T2[0:IN, 0:NS])
            nc.tensor.matmul(gc[:], w128c, s_c[:],
                             start=False, stop=True, skip_group_check=True)
            cand = wpool.tile([IN, NS], F16, tag="cand")
            nc.scalar.activation(cand[:], gc[:], ACT.Tanh)

            # ---- out = u*cand + (1-u)*h2 -----------------------------------
            e1 = wpool.tile([IN, NS], F16, tag="e1")
            nc.vector.tensor_mul(e1[:], u_sb[:], cand[:])
            outT = wpool.tile([IN, NS], F32, tag="outT")
            nc.vector.tensor_add(outT[:], e1[:], e2[:])
            nc.sync.dma_start(out_e[:], outT[:])
    nc.compile()
    return nc


def _f16(a):
    return np.ascontiguousarray(np.asarray(a, np.float16))


def prep_in_maps(x, h, query_vectors, adj, nodes_ind, W_r, b_r, W_u, b_u, W_c, b_c):
    x = np.asarray(x, np.float32)
    h = np.asarray(h, np.float32)
    q = np.asarray(query_vectors, np.float32)
    adj = np.asarray(adj, np.float32)
    ni = np.asarray(nodes_ind)
    assert np.array_equal(ni, np.arange(N)), "kernel assumes nodes_ind == arange(N)"

    xh = np.concatenate([x, h, np.zeros((N, 1), np.float32)], axis=-1)  # [N,130]
    xh_kt = xh.reshape(KT, 128, CI2).transpose(1, 0, 2)     # [128, KT, 130]

    Wr = np.asarray(W_r, np.float32)
    Wu = np.asarray(W_u, np.float32)
    Wc = np.asarray(W_c, np.float32)
    wdru = np.concatenate([Wr[:, :128, :], Wu[:, :128, :]], axis=2)  # [16,128,128]
    wdru = _f16(wdru.transpose(1, 0, 2).reshape(128, QD * 2 * IN))
    perm_c = list(range(65, CI)) + list(range(0, 64))                # [h2|x]
    wdc = Wc[:, perm_c, :]                                           # [16,128,64]
    wdc = _f16(wdc.transpose(1, 0, 2).reshape(128, QD * IN))

    sel17b = np.zeros((128, QD), np.float32)
    for jj in range(4):
        sel17b[32 * jj, :] = 1.0

    in_maps = []
    for c in range(NCORES):
        s = slice(c * NS, (c + 1) * NS)
        qs = q[s].T                                             # [16, 512]

        blobA = np.zeros((128, BW), np.float32)

        def put(name, arr):
            o = _OFF[name]
            blobA[0:arr.shape[0], o:o + arr.shape[1]] = arr

        put("qT", qs)
        put("bru", np.concatenate([np.asarray(b_r, np.float32),
                                   np.asarray(b_u, np.float32)], axis=1))
        put("bc", np.asarray(b_c, np.float32))
        put("sel17b", sel17b)
        put("w128ru", np.concatenate([Wr[:, 128, :], Wu[:, 128, :]], axis=1))
        put("w128c", Wc[:, 64, :])
        put("x64rep", np.tile(x[s, 64], (QD, 1)))
        put("xTc", x[s, 0:64].T)
        put("hT", h[s].T)

        adjT_kt = adj[s].T.reshape(KT, 128, NS).transpose(1, 0, 2)  # [128,KT,NS]
        strm = np.concatenate([xh_kt, adjT_kt], axis=2)             # [128,KT,CW]
        strm = _f16(strm.reshape(128, KT * CW))

        qbp = np.broadcast_to(qs[:, None, :], (QD, 128, NS))
        qbp = _f16(qbp.transpose(1, 0, 2).reshape(128, QD * NS))

        in_maps.append({
            "blobA": _f16(blobA),
            "strm": strm,
            "qbp": qbp,
            "wdru": wdru, "wdc": wdc,
        })
    return in_maps


def kernel(**inputs):
    from concourse.bass_utils import run_bass_kernel_spmd

    if "nc" not in _CACHE:
        _CACHE["nc"] = build_nc()
    nc = _CACHE["nc"]
    in_maps = prep_in_maps(**inputs)
    res = run_bass_kernel_spmd(nc, in_maps, core_ids=list(range(NCORES)))
    out = np.empty((N, IN), np.float32)
    for c in range(NCORES):
        out[c * NS:(c + 1) * NS, :] = res.results[c]["out"].T
    return out
